# revision 9
# baseline (speedup 1.0000x reference)
"""Trainium2 Bass kernel for nn_DecodeBox (YOLOv3-style box decode).

Contract: kernel(feat0, feat1, feat2) takes FULL inputs
  feat0 [32,255,19,19], feat1 [32,255,38,38], feat2 [32,255,76,76] (f32)
and returns the FULL output [32, 22743, 85] f32.

Strategy: pure data-parallel over batch (4 images per core, 8 cores).
Per core, per scale:
  - load per-(b,anchor) feature tiles with the 85 attrs on SBUF partitions
    ([85, HW], contiguous DRAM reads), sigmoid everything in place (ACT)
  - separately load a compact [48, HW] "box tile" holding channels 0..3 of
    every (b, anchor) pair; fix it batched: sigmoid+grid via one
    scalar_tensor_tensor, exp with per-partition ln(anchor/608) bias
  - PE-transpose 128-cell chunks ([85,128] -> PSUM [128,85]), DVE-copy the
    conf/cls columns to SBUF staging, stitch the 4 box columns from the
    transposed box tile, DMA rows (85 contiguous f32 each) to the output.
"""

import numpy as np

import concourse.bacc as bacc
import concourse.mybir as mybir
from concourse import masks, tile
from concourse.bass_utils import run_bass_kernel_spmd

F32 = mybir.dt.float32
AFT = mybir.ActivationFunctionType
ALU = mybir.AluOpType

N_CORES = 8
B_FULL = 32
B_LOCAL = B_FULL // N_CORES  # 4
ATTRS = 85
TOTAL_ROWS = 22743
GROUP = 6  # transpose chunks per PSUM group (6*85 = 510 f32 <= one 2KB bank)

ANCHORS = np.array(
    [[10, 13], [16, 30], [33, 23], [30, 61], [62, 45], [59, 119],
     [116, 90], [156, 198], [373, 326]], dtype=np.float32)
MASKS_ = [[6, 7, 8], [3, 4, 5], [0, 1, 2]]
SCALES = [(19, 0), (38, 1083), (76, 5415)]  # (grid G, output row offset)


def _chunk_starts(hw: int) -> list[int]:
    nfull = hw // 128
    starts = [i * 128 for i in range(nfull)]
    if hw % 128:
        starts.append(hw - 128)  # shifted last chunk; overlap rows rewritten
    return starts


def _groups(starts: list[int]) -> list[list[int]]:
    return [starts[i:i + GROUP] for i in range(0, len(starts), GROUP)]


def _runs(grp: list[int]):
    """Split a chunk group into maximal stride-128 runs -> (qoff, n, start0)."""
    runs, q = [], 0
    while q < len(grp):
        n = 1
        while q + n < len(grp) and grp[q + n] == grp[q] + 128 * n:
            n += 1
        runs.append((q, n, grp[q]))
        q += n
    return runs


def host_consts():
    """grid{s} [24,HW] = per-cell (x|y)/G; bias{s} [48,1] rows 24:48 =
    ln(anchor/608) keyed (k-2)*12 + b*3 + a."""
    out = {}
    for s, (g, _off) in enumerate(SCALES):
        hw = g * g
        grid = np.empty((24, hw), np.float32)
        grid[0:12] = (np.arange(hw, dtype=np.float32) % g) / g
        grid[12:24] = (np.arange(hw, dtype=np.float32) // g) / g
        bias = np.zeros((24, 1), np.float32)
        for k in range(2):
            for j in range(12):
                a = j % 3
                bias[k * 12 + j, 0] = np.log(ANCHORS[MASKS_[s][a]][k] / 608.0)
        out[f"grid{s}"] = grid
        out[f"bias{s}"] = bias
    return out


def build_nc():
    nc = bacc.Bacc("TRN2", target_bir_lowering=False, debug=False,
                   num_devices=N_CORES)
    feats = []
    grids = []
    biases = []
    for s, (g, _off) in enumerate(SCALES):
        feats.append(nc.dram_tensor(f"feat{s}", [B_LOCAL, 255, g, g], F32,
                                    kind="ExternalInput").ap())
        grids.append(nc.dram_tensor(f"grid{s}", [24, g * g], F32,
                                    kind="ExternalInput").ap())
        biases.append(nc.dram_tensor(f"bias{s}", [24, 1], F32,
                                     kind="ExternalInput").ap())
    out = nc.dram_tensor("out", [B_LOCAL, TOTAL_ROWS, ATTRS], F32,
                         kind="ExternalOutput").ap()

    with tile.TileContext(nc) as tc:
        with (
            tc.tile_pool(name="const", bufs=1) as const_pool,
            tc.tile_pool(name="box", bufs=1) as box_pool,
            tc.tile_pool(name="unit", bufs=3) as unit_pool,
            tc.tile_pool(name="bstage", bufs=12) as bstage_pool,
            tc.tile_pool(name="stage", bufs=6) as stage_pool,
            tc.tile_pool(name="pmain", bufs=4, space="PSUM") as psum_main,
            tc.tile_pool(name="pbox", bufs=2, space="PSUM") as psum_box,
        ):
            ident = const_pool.tile([128, 128], F32, tag="ident")
            masks.make_identity(nc, ident[:])

            grid_t, bias_t, xy_t, wh_t = {}, {}, {}, {}
            for s, (g, _off) in enumerate(SCALES):
                hw = g * g
                grid_t[s] = const_pool.tile([24, hw], F32, tag=f"grid{s}", name=f"grid_t{s}")
                nc.sync.dma_start(grid_t[s][:], grids[s][:])
                bias_t[s] = const_pool.tile([24, 1], F32, tag=f"bias{s}", name=f"bias_t{s}")
                nc.sync.dma_start(bias_t[s][:], biases[s][:])
                # box tiles: partition p = k*12 + b*3 + a (k in 0..1 each)
                src = feats[s].rearrange("b (a c) h w -> c b a (h w)", a=3)
                xy_t[s] = box_pool.tile([24, hw], F32, tag=f"boxxy{s}", name=f"xy_t{s}")
                nc.sync.dma_start(xy_t[s][:], src[0:2])
                wh_t[s] = box_pool.tile([24, hw], F32, tag=f"boxwh{s}", name=f"wh_t{s}")
                nc.sync.dma_start(wh_t[s][:], src[2:4])

            # cluster sigmoids together, then exps (ACT table loads cost 1.3us)
            for s in range(3):
                nc.scalar.activation(xy_t[s][:], xy_t[s][:], AFT.Sigmoid)
            for s in range(3):
                nc.scalar.activation(wh_t[s][:], wh_t[s][:], AFT.Exp,
                                     bias=bias_t[s][:])
            for s, (g, _off) in enumerate(SCALES):
                # xy = sigmoid(p)/G + grid/G
                nc.vector.scalar_tensor_tensor(
                    out=xy_t[s][:], in0=xy_t[s][:], scalar=1.0 / g,
                    in1=grid_t[s][:], op0=ALU.mult, op1=ALU.add)

            for s in (2, 1, 0):
                g, off = SCALES[s]
                hw = g * g
                groups = _groups(_chunk_starts(hw))

                # transpose the box tile once per chunk; stage to SBUF so the
                # per-(b,a) stitches below can read it (DMA can't read PSUM,
                # and holding all groups in PSUM would exhaust the 8 banks)
                bstages = []
                for grp in groups:
                    n = len(grp)
                    pb = psum_box.tile([128, 48 * n], F32, tag="pbox")
                    for q, st in enumerate(grp):
                        nc.tensor.transpose(pb[:, 48 * q:48 * q + 24],
                                            xy_t[s][:, st:st + 128],
                                            ident[0:24, 0:24])
                        nc.tensor.transpose(pb[:, 48 * q + 24:48 * (q + 1)],
                                            wh_t[s][:, st:st + 128],
                                            ident[0:24, 0:24])
                    bs = bstage_pool.tile([128, 48 * n], F32, tag="bstage")
                    nc.vector.tensor_copy(bs[:], pb[:])
                    bstages.append(bs)

                # units: s2 -> one [85,HW] tile per (b,a); s0/s1 -> one
                # [85,3*HW] tile per b covering all anchors
                if s == 2:
                    units = [((b, (a,)), feats[s][b, 85 * a:85 * (a + 1)]
                              .rearrange("c h w -> c (h w)"), hw)
                             for b in range(B_LOCAL) for a in range(3)]
                else:
                    units = [((b, (0, 1, 2)), feats[s][b]
                              .rearrange("(a c) h w -> c a (h w)", a=3), 3 * hw)
                             for b in range(B_LOCAL)]

                for (b, anchors), src_ap, ncols in units:
                    ut = unit_pool.tile([85, ncols], F32, tag="unit")
                    nc.sync.dma_start(ut[:], src_ap)
                    nc.scalar.activation(ut[:], ut[:], AFT.Sigmoid)
                    for ai, a in enumerate(anchors):
                        colbase = ai * hw
                        j = b * 3 + a
                        rbase = off + a * hw
                        for gi, grp in enumerate(groups):
                            n = len(grp)
                            pm = psum_main.tile([128, ATTRS * n], F32,
                                                tag="pmain")
                            for q, st in enumerate(grp):
                                nc.tensor.transpose(
                                    pm[:, ATTRS * q:ATTRS * (q + 1)],
                                    ut[:, colbase + st:colbase + st + 128],
                                    ident[0:85, 0:85])
                            stg = stage_pool.tile([128, ATTRS * n], F32,
                                                  tag="stage")
                            stg3 = stg[:].rearrange("p (q c) -> p q c", c=ATTRS)
                            pm3 = pm[:].rearrange("p (q c) -> p q c", c=ATTRS)
                            nc.vector.tensor_copy(stg3[:, :, 4:ATTRS],
                                                  pm3[:, :, 4:ATTRS])
                            bsrc = bstages[gi][:].rearrange(
                                "p (q k j) -> p q k j", k=4, j=12)
                            nc.vector.tensor_copy(stg3[:, :, 0:4],
                                                  bsrc[:, :, :, j])
                            for qoff, nrun, st0 in _runs(grp):
                                src = stg3[:, qoff:qoff + nrun, :]
                                dst = out[b, rbase + st0:
                                          rbase + st0 + nrun * 128, :] \
                                    .rearrange("(q p) c -> p q c", p=128)
                                nc.sync.dma_start(dst, src)
    nc.compile()
    return nc


_NC_CACHE = []


def _get_nc():
    if not _NC_CACHE:
        _NC_CACHE.append(build_nc())
    return _NC_CACHE[0]


def kernel(feat0, feat1, feat2):
    feats = [np.ascontiguousarray(np.asarray(f, dtype=np.float32))
             for f in (feat0, feat1, feat2)]
    assert feats[0].shape == (B_FULL, 255, 19, 19)
    assert feats[1].shape == (B_FULL, 255, 38, 38)
    assert feats[2].shape == (B_FULL, 255, 76, 76)

    consts = host_consts()
    nc = _get_nc()
    in_maps = []
    for c in range(N_CORES):
        m = dict(consts)
        for s in range(3):
            m[f"feat{s}"] = feats[s][c * B_LOCAL:(c + 1) * B_LOCAL]
        in_maps.append(m)

    res = run_bass_kernel_spmd(nc, in_maps, list(range(N_CORES)))
    return np.concatenate([res.results[c]["out"] for c in range(N_CORES)],
                          axis=0)


# revision 13
# speedup vs baseline: 6.1632x; 6.1632x over previous
"""Trainium2 Bass kernel for nn_DecodeBox (YOLOv3-style box decode).

Contract: kernel(feat0, feat1, feat2) takes FULL inputs
  feat0 [32,255,19,19], feat1 [32,255,38,38], feat2 [32,255,76,76] (f32)
and returns the FULL output [32, 22743, 85] f32.

Strategy: pure data-parallel over batch (4 images per core, 8 cores).
Per core, per scale:
  - load per-(b,anchor) feature tiles with the 85 attrs on SBUF partitions
    ([85, HW], contiguous DRAM reads), sigmoid everything in place (ACT)
  - separately load a compact [48, HW] "box tile" holding channels 0..3 of
    every (b, anchor) pair; fix it batched: sigmoid+grid via one
    scalar_tensor_tensor, exp with per-partition ln(anchor/608) bias
  - PE-transpose 128-cell chunks ([85,128] -> PSUM [128,85]), DVE-copy the
    conf/cls columns to SBUF staging, stitch the 4 box columns from the
    transposed box tile, DMA rows (85 contiguous f32 each) to the output.
"""

import numpy as np

import concourse.bacc as bacc
import concourse.mybir as mybir
from concourse import masks, tile
from concourse.bass_utils import run_bass_kernel_spmd

F32 = mybir.dt.float32
AFT = mybir.ActivationFunctionType
ALU = mybir.AluOpType

N_CORES = 8
B_FULL = 32
B_LOCAL = B_FULL // N_CORES  # 4
ATTRS = 85
TOTAL_ROWS = 22743
GROUP = 6  # transpose chunks per PSUM group (6*85 = 510 f32 <= one 2KB bank)
SGROUP = 4  # PSUM groups per staging tile / store (up to 24 chunks ~ 1MB)

ANCHORS = np.array(
    [[10, 13], [16, 30], [33, 23], [30, 61], [62, 45], [59, 119],
     [116, 90], [156, 198], [373, 326]], dtype=np.float32)
MASKS_ = [[6, 7, 8], [3, 4, 5], [0, 1, 2]]
SCALES = [(19, 0), (38, 1083), (76, 5415)]  # (grid G, output row offset)


def _chunk_starts(hw: int) -> list[int]:
    nfull = hw // 128
    starts = [i * 128 for i in range(nfull)]
    if hw % 128:
        starts.append(hw - 128)  # shifted last chunk; overlap rows rewritten
    return starts


def _groups(starts: list[int]) -> list[list[int]]:
    return [starts[i:i + GROUP] for i in range(0, len(starts), GROUP)]


def _runs(grp: list[int]):
    """Split a chunk group into maximal stride-128 runs -> (qoff, n, start0)."""
    runs, q = [], 0
    while q < len(grp):
        n = 1
        while q + n < len(grp) and grp[q + n] == grp[q] + 128 * n:
            n += 1
        runs.append((q, n, grp[q]))
        q += n
    return runs


def host_consts():
    """grid{s} [24,HW] = per-cell (x|y)/G; bias{s} [48,1] rows 24:48 =
    ln(anchor/608) keyed (k-2)*12 + b*3 + a."""
    out = {}
    for s, (g, _off) in enumerate(SCALES):
        hw = g * g
        grid = np.empty((24, hw), np.float32)
        grid[0:12] = (np.arange(hw, dtype=np.float32) % g) / g
        grid[12:24] = (np.arange(hw, dtype=np.float32) // g) / g
        bias = np.zeros((24, 1), np.float32)
        for k in range(2):
            for j in range(12):
                a = j % 3
                bias[k * 12 + j, 0] = np.log(ANCHORS[MASKS_[s][a]][k] / 608.0)
        out[f"grid{s}"] = grid
        out[f"bias{s}"] = bias
    return out


def build_nc():
    nc = bacc.Bacc("TRN2", target_bir_lowering=False, debug=False,
                   num_devices=N_CORES)
    feats = []
    grids = []
    biases = []
    for s, (g, _off) in enumerate(SCALES):
        feats.append(nc.dram_tensor(f"feat{s}", [B_LOCAL, 255, g, g], F32,
                                    kind="ExternalInput").ap())
        grids.append(nc.dram_tensor(f"grid{s}", [24, g * g], F32,
                                    kind="ExternalInput").ap())
        biases.append(nc.dram_tensor(f"bias{s}", [24, 1], F32,
                                     kind="ExternalInput").ap())
    out = nc.dram_tensor("out", [B_LOCAL, TOTAL_ROWS, ATTRS], F32,
                         kind="ExternalOutput").ap()

    with tile.TileContext(nc) as tc:
        with (
            tc.tile_pool(name="const", bufs=1) as const_pool,
            tc.tile_pool(name="box", bufs=1) as box_pool,
            tc.tile_pool(name="unit", bufs=3) as unit_pool,
            tc.tile_pool(name="bstage", bufs=12) as bstage_pool,
            tc.tile_pool(name="stage", bufs=3) as stage_pool,
            tc.tile_pool(name="pmain", bufs=4, space="PSUM") as psum_main,
            tc.tile_pool(name="pbox", bufs=2, space="PSUM") as psum_box,
        ):
            ident = const_pool.tile([128, 128], F32, tag="ident")
            masks.make_identity(nc, ident[:])

            grid_t, bias_t, xy_t, wh_t = {}, {}, {}, {}
            for s, (g, _off) in enumerate(SCALES):
                hw = g * g
                grid_t[s] = const_pool.tile([24, hw], F32, tag=f"grid{s}", name=f"grid_t{s}")
                nc.gpsimd.dma_start(grid_t[s][:], grids[s][:])
                bias_t[s] = const_pool.tile([24, 1], F32, tag=f"bias{s}", name=f"bias_t{s}")
                nc.gpsimd.dma_start(bias_t[s][:], biases[s][:])
                # box tiles: partition p = k*12 + b*3 + a (k in 0..1 each)
                src = feats[s].rearrange("b (a c) h w -> c b a (h w)", a=3)
                xy_t[s] = box_pool.tile([24, hw], F32, tag=f"boxxy{s}", name=f"xy_t{s}")
                nc.gpsimd.dma_start(xy_t[s][:], src[0:2])
                wh_t[s] = box_pool.tile([24, hw], F32, tag=f"boxwh{s}", name=f"wh_t{s}")
                nc.gpsimd.dma_start(wh_t[s][:], src[2:4])

            # cluster sigmoids together, then exps (ACT table loads cost 1.3us)
            for s in range(3):
                nc.scalar.activation(xy_t[s][:], xy_t[s][:], AFT.Sigmoid)
            for s in range(3):
                nc.scalar.activation(wh_t[s][:], wh_t[s][:], AFT.Exp,
                                     bias=bias_t[s][:])
            for s, (g, _off) in enumerate(SCALES):
                # xy = sigmoid(p)/G + grid/G
                nc.vector.scalar_tensor_tensor(
                    out=xy_t[s][:], in0=xy_t[s][:], scalar=1.0 / g,
                    in1=grid_t[s][:], op0=ALU.mult, op1=ALU.add)

            for s in (2, 1, 0):
                g, off = SCALES[s]
                hw = g * g
                groups = _groups(_chunk_starts(hw))

                # transpose the box tile once per chunk; stage to SBUF so the
                # per-(b,a) stitches below can read it (DMA can't read PSUM,
                # and holding all groups in PSUM would exhaust the 8 banks)
                bstages = []
                for grp in groups:
                    n = len(grp)
                    pb = psum_box.tile([128, 48 * n], F32, tag="pbox")
                    for q, st in enumerate(grp):
                        nc.tensor.transpose(pb[:, 48 * q:48 * q + 24],
                                            xy_t[s][:, st:st + 128],
                                            ident[0:24, 0:24])
                        nc.tensor.transpose(pb[:, 48 * q + 24:48 * (q + 1)],
                                            wh_t[s][:, st:st + 128],
                                            ident[0:24, 0:24])
                    bs = bstage_pool.tile([128, 48 * n], F32, tag="bstage")
                    nc.vector.tensor_copy(bs[:], pb[:])
                    bstages.append(bs)

                # units: s2 -> one [85,HW] tile per (b,a); s0/s1 -> one
                # [85,3*HW] tile per b covering all anchors
                if s == 2:
                    units = [((b, (a,)), feats[s][b, 85 * a:85 * (a + 1)]
                              .rearrange("c h w -> c (h w)"), hw)
                             for b in range(B_LOCAL) for a in range(3)]
                else:
                    units = [((b, (0, 1, 2)), feats[s][b]
                              .rearrange("(a c) h w -> c a (h w)", a=3), 3 * hw)
                             for b in range(B_LOCAL)]

                # store groups: up to SGROUP consecutive PSUM groups share one
                # staging tile and one (or two, around the shifted chunk) DMA
                sgroups = [list(range(i, min(i + SGROUP, len(groups))))
                           for i in range(0, len(groups), SGROUP)]

                for (b, anchors), src_ap, ncols in units:
                    ut = unit_pool.tile([85, ncols], F32, tag="unit")
                    nc.sync.dma_start(ut[:], src_ap)
                    nc.scalar.activation(ut[:], ut[:], AFT.Sigmoid)
                    for ai, a in enumerate(anchors):
                        colbase = ai * hw
                        j = b * 3 + a
                        rbase = off + a * hw
                        for sgi in sgroups:
                            sg_chunks = [st for gi in sgi for st in groups[gi]]
                            nsg = len(sg_chunks)
                            stg = stage_pool.tile([128, ATTRS * nsg], F32,
                                                  tag="stage")
                            stg3 = stg[:].rearrange("p (q c) -> p q c", c=ATTRS)
                            qbase = 0
                            for gi in sgi:
                                grp = groups[gi]
                                n = len(grp)
                                pm = psum_main.tile([128, ATTRS * n], F32,
                                                    tag="pmain")
                                for q, st in enumerate(grp):
                                    nc.tensor.transpose(
                                        pm[:, ATTRS * q:ATTRS * (q + 1)],
                                        ut[:, colbase + st:colbase + st + 128],
                                        ident[0:85, 0:85])
                                pm3 = pm[:].rearrange("p (q c) -> p q c",
                                                      c=ATTRS)
                                sl = stg3[:, qbase:qbase + n, :]
                                nc.vector.tensor_copy(sl[:, :, 4:ATTRS],
                                                      pm3[:, :, 4:ATTRS])
                                bsrc = bstages[gi][:].rearrange(
                                    "p (q k j) -> p q k j", k=4, j=12)
                                nc.vector.tensor_copy(sl[:, :, 0:4],
                                                      bsrc[:, :, :, j])
                                qbase += n
                            for qoff, nrun, st0 in _runs(sg_chunks):
                                src = stg3[:, qoff:qoff + nrun, :]
                                dst = out[b, rbase + st0:
                                          rbase + st0 + nrun * 128, :] \
                                    .rearrange("(q p) c -> p q c", p=128)
                                nc.scalar.dma_start(dst, src)
    nc.compile()
    return nc


_NC_CACHE = []


def _get_nc():
    if not _NC_CACHE:
        _NC_CACHE.append(build_nc())
    return _NC_CACHE[0]


def kernel(feat0, feat1, feat2):
    feats = [np.ascontiguousarray(np.asarray(f, dtype=np.float32))
             for f in (feat0, feat1, feat2)]
    assert feats[0].shape == (B_FULL, 255, 19, 19)
    assert feats[1].shape == (B_FULL, 255, 38, 38)
    assert feats[2].shape == (B_FULL, 255, 76, 76)

    consts = host_consts()
    nc = _get_nc()
    in_maps = []
    for c in range(N_CORES):
        m = dict(consts)
        for s in range(3):
            m[f"feat{s}"] = feats[s][c * B_LOCAL:(c + 1) * B_LOCAL]
        in_maps.append(m)

    res = run_bass_kernel_spmd(nc, in_maps, list(range(N_CORES)))
    return np.concatenate([res.results[c]["out"] for c in range(N_CORES)],
                          axis=0)


# revision 15
# speedup vs baseline: 12.0840x; 1.9607x over previous
"""Trainium2 Bass kernel for nn_DecodeBox (YOLOv3-style box decode).

Contract: kernel(feat0, feat1, feat2) takes FULL inputs
  feat0 [32,255,19,19], feat1 [32,255,38,38], feat2 [32,255,76,76] (f32)
and returns the FULL output [32, 22743, 85] f32.

Strategy: pure data-parallel over batch (4 images per core, 8 cores).
Per core, per scale:
  - load per-(b,anchor) feature tiles with the 85 attrs on SBUF partitions
    ([85, HW], contiguous DRAM reads), sigmoid everything in place (ACT)
  - separately load a compact [48, HW] "box tile" holding channels 0..3 of
    every (b, anchor) pair; fix it batched: sigmoid+grid via one
    scalar_tensor_tensor, exp with per-partition ln(anchor/608) bias
  - PE-transpose 128-cell chunks ([85,128] -> PSUM [128,85]), DVE-copy the
    conf/cls columns to SBUF staging, stitch the 4 box columns from the
    transposed box tile, DMA rows (85 contiguous f32 each) to the output.
"""

import numpy as np

import concourse.bacc as bacc
import concourse.mybir as mybir
from concourse import masks, tile
from concourse.bass_utils import run_bass_kernel_spmd

F32 = mybir.dt.float32
AFT = mybir.ActivationFunctionType
ALU = mybir.AluOpType

N_CORES = 8
B_FULL = 32
B_LOCAL = B_FULL // N_CORES  # 4
ATTRS = 85
TOTAL_ROWS = 22743
GROUP = 6  # transpose chunks per PSUM group (6*85 = 510 f32 <= one 2KB bank)
SGROUP = 4  # PSUM groups per staging tile / store (up to 24 chunks ~ 1MB)

ANCHORS = np.array(
    [[10, 13], [16, 30], [33, 23], [30, 61], [62, 45], [59, 119],
     [116, 90], [156, 198], [373, 326]], dtype=np.float32)
MASKS_ = [[6, 7, 8], [3, 4, 5], [0, 1, 2]]
SCALES = [(19, 0), (38, 1083), (76, 5415)]  # (grid G, output row offset)


def _chunk_starts(hw: int) -> list[int]:
    nfull = hw // 128
    starts = [i * 128 for i in range(nfull)]
    if hw % 128:
        starts.append(hw - 128)  # shifted last chunk; overlap rows rewritten
    return starts


def _groups(starts: list[int]) -> list[list[int]]:
    return [starts[i:i + GROUP] for i in range(0, len(starts), GROUP)]


def _runs(grp: list[int]):
    """Split a chunk group into maximal stride-128 runs -> (qoff, n, start0)."""
    runs, q = [], 0
    while q < len(grp):
        n = 1
        while q + n < len(grp) and grp[q + n] == grp[q] + 128 * n:
            n += 1
        runs.append((q, n, grp[q]))
        q += n
    return runs


def host_consts():
    """grid{s} [24,HW] = per-cell (x|y)/G; bias{s} [48,1] rows 24:48 =
    ln(anchor/608) keyed (k-2)*12 + b*3 + a."""
    out = {}
    for s, (g, _off) in enumerate(SCALES):
        hw = g * g
        grid = np.empty((24, hw), np.float32)
        grid[0:12] = (np.arange(hw, dtype=np.float32) % g) / g
        grid[12:24] = (np.arange(hw, dtype=np.float32) // g) / g
        bias = np.zeros((24, 1), np.float32)
        for k in range(2):
            for j in range(12):
                a = j % 3
                bias[k * 12 + j, 0] = np.log(ANCHORS[MASKS_[s][a]][k] / 608.0)
        out[f"grid{s}"] = grid
        out[f"bias{s}"] = bias
    return out


def build_nc(repeat: int = 1):
    nc = bacc.Bacc("TRN2", target_bir_lowering=False, debug=False,
                   num_devices=N_CORES)
    feats = []
    grids = []
    biases = []
    for s, (g, _off) in enumerate(SCALES):
        feats.append(nc.dram_tensor(f"feat{s}", [B_LOCAL, 255, g, g], F32,
                                    kind="ExternalInput").ap())
        grids.append(nc.dram_tensor(f"grid{s}", [24, g * g], F32,
                                    kind="ExternalInput").ap())
        biases.append(nc.dram_tensor(f"bias{s}", [24, 1], F32,
                                     kind="ExternalInput").ap())
    out = nc.dram_tensor("out", [B_LOCAL, TOTAL_ROWS, ATTRS], F32,
                         kind="ExternalOutput").ap()

    with tile.TileContext(nc) as tc:
        with (
            tc.tile_pool(name="const", bufs=1) as const_pool,
            tc.tile_pool(name="box", bufs=1) as box_pool,
            tc.tile_pool(name="unit", bufs=3) as unit_pool,
            tc.tile_pool(name="bstage", bufs=12) as bstage_pool,
            tc.tile_pool(name="stage", bufs=3) as stage_pool,
            tc.tile_pool(name="pmain", bufs=4, space="PSUM") as psum_main,
            tc.tile_pool(name="pbox", bufs=2, space="PSUM") as psum_box,
        ):
            ident = const_pool.tile([128, 128], F32, tag="ident")
            masks.make_identity(nc, ident[:])

            grid_t = {}
            bias_t = {}
            for s, (g, _off) in enumerate(SCALES):
                hw = g * g
                grid_t[s] = const_pool.tile([24, hw], F32, tag=f"grid{s}", name=f"grid_t{s}")
                nc.gpsimd.dma_start(grid_t[s][:], grids[s][:])
                bias_t[s] = const_pool.tile([24, 1], F32, tag=f"bias{s}", name=f"bias_t{s}")
                nc.gpsimd.dma_start(bias_t[s][:], biases[s][:])

            for _rep in range(repeat):
              xy_t, wh_t = {}, {}
              for s, (g, _off) in enumerate(SCALES):
                hw = g * g
                # box tiles: partition p = k*12 + b*3 + a (k in 0..1 each)
                src = feats[s].rearrange("b (a c) h w -> c b a (h w)", a=3)
                xy_t[s] = box_pool.tile([24, hw], F32, tag=f"boxxy{s}", name=f"xy_t{s}")
                nc.gpsimd.dma_start(xy_t[s][:], src[0:2])
                wh_t[s] = box_pool.tile([24, hw], F32, tag=f"boxwh{s}", name=f"wh_t{s}")
                nc.gpsimd.dma_start(wh_t[s][:], src[2:4])

              # cluster sigmoids together, then exps (ACT table loads cost 1.3us)
              for s in range(3):
                nc.scalar.activation(xy_t[s][:], xy_t[s][:], AFT.Sigmoid)
              for s in range(3):
                nc.scalar.activation(wh_t[s][:], wh_t[s][:], AFT.Exp,
                                     bias=bias_t[s][:])
              for s, (g, _off) in enumerate(SCALES):
                # xy = sigmoid(p)/G + grid/G
                nc.vector.scalar_tensor_tensor(
                    out=xy_t[s][:], in0=xy_t[s][:], scalar=1.0 / g,
                    in1=grid_t[s][:], op0=ALU.mult, op1=ALU.add)

              for s in (2, 1, 0):
                g, off = SCALES[s]
                hw = g * g
                groups = _groups(_chunk_starts(hw))

                # transpose the box tile once per chunk; stage to SBUF so the
                # per-(b,a) stitches below can read it (DMA can't read PSUM,
                # and holding all groups in PSUM would exhaust the 8 banks)
                bstages = []
                for grp in groups:
                    n = len(grp)
                    pb = psum_box.tile([128, 48 * n], F32, tag="pbox")
                    for q, st in enumerate(grp):
                        nc.tensor.transpose(pb[:, 48 * q:48 * q + 24],
                                            xy_t[s][:, st:st + 128],
                                            ident[0:24, 0:24])
                        nc.tensor.transpose(pb[:, 48 * q + 24:48 * (q + 1)],
                                            wh_t[s][:, st:st + 128],
                                            ident[0:24, 0:24])
                    bs = bstage_pool.tile([128, 48 * n], F32, tag="bstage")
                    nc.vector.tensor_copy(bs[:], pb[:])
                    bstages.append(bs)

                # units: s2 -> one [85,HW] tile per (b,a); s0/s1 -> one
                # [85,3*HW] tile per b covering all anchors
                if s == 2:
                    units = [((b, (a,)), feats[s][b, 85 * a:85 * (a + 1)]
                              .rearrange("c h w -> c (h w)"), hw)
                             for b in range(B_LOCAL) for a in range(3)]
                else:
                    units = [((b, (0, 1, 2)), feats[s][b]
                              .rearrange("(a c) h w -> c a (h w)", a=3), 3 * hw)
                             for b in range(B_LOCAL)]

                # store groups: up to SGROUP consecutive PSUM groups share one
                # staging tile and one (or two, around the shifted chunk) DMA
                sgroups = [list(range(i, min(i + SGROUP, len(groups))))
                           for i in range(0, len(groups), SGROUP)]

                for (b, anchors), src_ap, ncols in units:
                    ut = unit_pool.tile([85, ncols], F32, tag="unit")
                    nc.sync.dma_start(ut[:], src_ap)
                    nc.scalar.activation(ut[:], ut[:], AFT.Sigmoid)
                    for ai, a in enumerate(anchors):
                        colbase = ai * hw
                        j = b * 3 + a
                        rbase = off + a * hw
                        for sgi in sgroups:
                            sg_chunks = [st for gi in sgi for st in groups[gi]]
                            nsg = len(sg_chunks)
                            stg = stage_pool.tile([128, ATTRS * nsg], F32,
                                                  tag="stage")
                            stg3 = stg[:].rearrange("p (q c) -> p q c", c=ATTRS)
                            qbase = 0
                            for gi in sgi:
                                grp = groups[gi]
                                n = len(grp)
                                pm = psum_main.tile([128, ATTRS * n], F32,
                                                    tag="pmain")
                                for q, st in enumerate(grp):
                                    nc.tensor.transpose(
                                        pm[:, ATTRS * q:ATTRS * (q + 1)],
                                        ut[:, colbase + st:colbase + st + 128],
                                        ident[0:85, 0:85])
                                pm3 = pm[:].rearrange("p (q c) -> p q c",
                                                      c=ATTRS)
                                sl = stg3[:, qbase:qbase + n, :]
                                nc.vector.tensor_copy(sl[:, :, 4:ATTRS],
                                                      pm3[:, :, 4:ATTRS])
                                bsrc = bstages[gi][:].rearrange(
                                    "p (q k j) -> p q k j", k=4, j=12)
                                nc.vector.tensor_copy(sl[:, :, 0:4],
                                                      bsrc[:, :, :, j])
                                qbase += n
                            for qoff, nrun, st0 in _runs(sg_chunks):
                                src = stg3[:, qoff:qoff + nrun, :]
                                dst = out[b, rbase + st0:
                                          rbase + st0 + nrun * 128, :] \
                                    .rearrange("(q p) c -> p q c", p=128)
                                nc.scalar.dma_start(dst, src)
    nc.compile()
    return nc


_NC_CACHE = []


def _get_nc():
    if not _NC_CACHE:
        _NC_CACHE.append(build_nc())
    return _NC_CACHE[0]


def kernel(feat0, feat1, feat2):
    feats = [np.ascontiguousarray(np.asarray(f, dtype=np.float32))
             for f in (feat0, feat1, feat2)]
    assert feats[0].shape == (B_FULL, 255, 19, 19)
    assert feats[1].shape == (B_FULL, 255, 38, 38)
    assert feats[2].shape == (B_FULL, 255, 76, 76)

    consts = host_consts()
    nc = _get_nc()
    in_maps = []
    for c in range(N_CORES):
        m = dict(consts)
        for s in range(3):
            m[f"feat{s}"] = feats[s][c * B_LOCAL:(c + 1) * B_LOCAL]
        in_maps.append(m)

    res = run_bass_kernel_spmd(nc, in_maps, list(range(N_CORES)))
    return np.concatenate([res.results[c]["out"] for c in range(N_CORES)],
                          axis=0)


# revision 16
# speedup vs baseline: 36.2232x; 2.9976x over previous
"""Trainium2 Bass kernel for nn_DecodeBox (YOLOv3-style box decode).

Contract: kernel(feat0, feat1, feat2) takes FULL inputs
  feat0 [32,255,19,19], feat1 [32,255,38,38], feat2 [32,255,76,76] (f32)
and returns the FULL output [32, 22743, 85] f32.

Strategy: pure data-parallel over batch (4 images per core, 8 cores).
Per core, per scale:
  - load per-(b,anchor) feature tiles with the 85 attrs on SBUF partitions
    ([85, HW], contiguous DRAM reads), sigmoid everything in place (ACT)
  - separately load compact [24, HW] xy/wh tiles (channels 0..3 of every
    (b, anchor) pair); fix them batched: sigmoid+grid via one
    scalar_tensor_tensor, exp with per-partition ln(anchor/608) bias
  - PE-transpose cells into PSUM, DVE-copy conf/cls columns to SBUF
    staging, stitch the 4 box columns from the transposed box tiles, DMA
    out. Bulk cells go through "quad blocks": 4 stride-4 transposes per
    512 cells so each SBUF partition holds 4 consecutive output rows ->
    1360B DMA store elements (>=512B avoids HBM read-modify-write).
    Leftover cells (<512 per (b,a)) use single 128-cell chunks (340B).
"""

import numpy as np

import concourse.bacc as bacc
import concourse.mybir as mybir
from concourse import masks, tile
from concourse.bass_utils import run_bass_kernel_spmd

F32 = mybir.dt.float32
AFT = mybir.ActivationFunctionType
ALU = mybir.AluOpType

N_CORES = 8
B_FULL = 32
B_LOCAL = B_FULL // N_CORES  # 4
ATTRS = 85
TOTAL_ROWS = 22743
SGQ = 4    # quad blocks per staging tile / store  (4*512 rows ~ 700KB)
GROUP = 6  # single chunks per PSUM group in the tail path

ANCHORS = np.array(
    [[10, 13], [16, 30], [33, 23], [30, 61], [62, 45], [59, 119],
     [116, 90], [156, 198], [373, 326]], dtype=np.float32)
MASKS_ = [[6, 7, 8], [3, 4, 5], [0, 1, 2]]
SCALES = [(19, 0), (38, 1083), (76, 5415)]  # (grid G, output row offset)


def _layout(hw: int):
    """-> (quad_starts, tail_starts): quads cover 512 cells each; tail is
    single 128-cell chunks, the last one shifted to end at hw (overlap rows
    are double-written with identical values)."""
    nq = hw // 512
    quads = [512 * i for i in range(nq)]
    rem = hw - 512 * nq
    tails = [512 * nq + 128 * i for i in range(rem // 128)]
    if hw % 128:
        tails.append(hw - 128)
    return quads, tails


def _groups(starts, n):
    return [starts[i:i + n] for i in range(0, len(starts), n)]


def _runs(grp, stride):
    runs, q = [], 0
    while q < len(grp):
        n = 1
        while q + n < len(grp) and grp[q + n] == grp[q] + stride * n:
            n += 1
        runs.append((q, n, grp[q]))
        q += n
    return runs


def host_consts():
    """grid{s} [24,HW] rows 0:12 = cellx/G, rows 12:24 = celly/G;
    bias{s} [24,1] row k*12 + b*3 + a = ln(anchor_dim_k/608)."""
    out = {}
    for s, (g, _off) in enumerate(SCALES):
        hw = g * g
        grid = np.empty((24, hw), np.float32)
        grid[0:12] = (np.arange(hw, dtype=np.float32) % g) / g
        grid[12:24] = (np.arange(hw, dtype=np.float32) // g) / g
        bias = np.zeros((24, 1), np.float32)
        for k in range(2):
            for j in range(12):
                a = j % 3
                bias[k * 12 + j, 0] = np.log(ANCHORS[MASKS_[s][a]][k] / 608.0)
        out[f"grid{s}"] = grid
        out[f"bias{s}"] = bias
    return out


def build_nc(repeat: int = 1):
    nc = bacc.Bacc("TRN2", target_bir_lowering=False, debug=False,
                   num_devices=N_CORES)
    feats, grids, biases = [], [], []
    for s, (g, _off) in enumerate(SCALES):
        feats.append(nc.dram_tensor(f"feat{s}", [B_LOCAL, 255, g, g], F32,
                                    kind="ExternalInput").ap())
        grids.append(nc.dram_tensor(f"grid{s}", [24, g * g], F32,
                                    kind="ExternalInput").ap())
        biases.append(nc.dram_tensor(f"bias{s}", [24, 1], F32,
                                     kind="ExternalInput").ap())
    out = nc.dram_tensor("out", [B_LOCAL, TOTAL_ROWS, ATTRS], F32,
                         kind="ExternalOutput").ap()

    with tile.TileContext(nc) as tc:
        with (
            tc.tile_pool(name="const", bufs=1) as const_pool,
            tc.tile_pool(name="box", bufs=1) as box_pool,
            tc.tile_pool(name="unit", bufs=3) as unit_pool,
            tc.tile_pool(name="bstage", bufs=16) as bstage_pool,
            tc.tile_pool(name="stage", bufs=4) as stage_pool,
            tc.tile_pool(name="pmain", bufs=6, space="PSUM") as psum_main,
            tc.tile_pool(name="pbox", bufs=2, space="PSUM") as psum_box,
        ):
            ident = const_pool.tile([128, 128], F32, tag="ident")
            masks.make_identity(nc, ident[:])

            grid_t, bias_t = {}, {}
            for s, (g, _off) in enumerate(SCALES):
                hw = g * g
                grid_t[s] = const_pool.tile([24, hw], F32, tag=f"grid{s}",
                                            name=f"grid_t{s}")
                nc.gpsimd.dma_start(grid_t[s][:], grids[s][:])
                bias_t[s] = const_pool.tile([24, 1], F32, tag=f"bias{s}",
                                            name=f"bias_t{s}")
                nc.gpsimd.dma_start(bias_t[s][:], biases[s][:])

            for _rep in range(repeat):
                _emit_pass(nc, tc, feats, out, grid_t, bias_t, ident,
                           box_pool, unit_pool, bstage_pool, stage_pool,
                           psum_main, psum_box)
    nc.compile()
    return nc


def _strided_cols(ap, start, espan, e):
    """ap[:, start + e :: espan] over espan*128 cells -> [P, 128] stride-espan
    column slice starting at cell start+e."""
    return ap[:, start:start + espan * 128] \
        .rearrange("p (f e) -> p e f", e=espan)[:, e, :]


def _emit_pass(nc, tc, feats, out, grid_t, bias_t, ident,
               box_pool, unit_pool, bstage_pool, stage_pool,
               psum_main, psum_box):
    xy_t, wh_t = {}, {}
    for s, (g, _off) in enumerate(SCALES):
        hw = g * g
        # box tiles: partition p = k*12 + b*3 + a (k in 0..1 each)
        src = feats[s].rearrange("b (a c) h w -> c b a (h w)", a=3)
        xy_t[s] = box_pool.tile([24, hw], F32, tag=f"boxxy{s}",
                                name=f"xy_t{s}")
        nc.gpsimd.dma_start(xy_t[s][:], src[0:2])
        wh_t[s] = box_pool.tile([24, hw], F32, tag=f"boxwh{s}",
                                name=f"wh_t{s}")
        nc.gpsimd.dma_start(wh_t[s][:], src[2:4])

    # cluster sigmoids together, then exps (ACT table loads cost 1.3us)
    for s in range(3):
        nc.scalar.activation(xy_t[s][:], xy_t[s][:], AFT.Sigmoid)
    for s in range(3):
        nc.scalar.activation(wh_t[s][:], wh_t[s][:], AFT.Exp,
                             bias=bias_t[s][:])
    for s, (g, _off) in enumerate(SCALES):
        # xy = sigmoid(p)/G + grid/G
        nc.vector.scalar_tensor_tensor(
            out=xy_t[s][:], in0=xy_t[s][:], scalar=1.0 / g,
            in1=grid_t[s][:], op0=ALU.mult, op1=ALU.add)

    for s in (2, 1, 0):
        g, off = SCALES[s]
        hw = g * g
        quads, tails = _layout(hw)
        tail_groups = _groups(tails, GROUP)

        # blocks: ("q", start) quad of 512 cells -> 4 stride-4 transposes
        #         ("t", [starts]) tail group of single chunks
        blocks = [("q", st) for st in quads] + \
                 [("t", grp) for grp in tail_groups]

        # transpose the box tiles once per block; stage to SBUF (DMA can't
        # read PSUM and PSUM can't hold all groups of a scale)
        bstages = []
        for kind, blk in blocks:
            if kind == "q":
                pb = psum_box.tile([128, 192], F32, tag="pbox")
                for e in range(4):
                    nc.tensor.transpose(
                        pb[:, 48 * e:48 * e + 24],
                        _strided_cols(xy_t[s][:], blk, 4, e),
                        ident[0:24, 0:24])
                    nc.tensor.transpose(
                        pb[:, 48 * e + 24:48 * (e + 1)],
                        _strided_cols(wh_t[s][:], blk, 4, e),
                        ident[0:24, 0:24])
                ncols = 192
            else:
                n = len(blk)
                ncols = 48 * n
                pb = psum_box.tile([128, ncols], F32, tag="pbox")
                for q, st in enumerate(blk):
                    nc.tensor.transpose(pb[:, 48 * q:48 * q + 24],
                                        xy_t[s][:, st:st + 128],
                                        ident[0:24, 0:24])
                    nc.tensor.transpose(pb[:, 48 * q + 24:48 * (q + 1)],
                                        wh_t[s][:, st:st + 128],
                                        ident[0:24, 0:24])
            bs = bstage_pool.tile([128, ncols], F32, tag="bstage")
            nc.vector.tensor_copy(bs[:], pb[:])
            bstages.append(bs)

        # units: s2 -> one [85,HW] tile per (b,a); s0/s1 -> one [85,3*HW]
        # tile per b covering all anchors
        if s == 2:
            units = [((b, (a,)), feats[s][b, 85 * a:85 * (a + 1)]
                      .rearrange("c h w -> c (h w)"), hw)
                     for b in range(B_LOCAL) for a in range(3)]
        else:
            units = [((b, (0, 1, 2)), feats[s][b]
                      .rearrange("(a c) h w -> c a (h w)", a=3), 3 * hw)
                     for b in range(B_LOCAL)]

        nquad_blocks = len(quads)
        qsgroups = _groups(list(range(nquad_blocks)), SGQ)

        for (b, anchors), src_ap, ncols_u in units:
            ut = unit_pool.tile([85, ncols_u], F32, tag="unit")
            nc.sync.dma_start(ut[:], src_ap)
            nc.scalar.activation(ut[:], ut[:], AFT.Sigmoid)
            for ai, a in enumerate(anchors):
                colbase = ai * hw
                j = b * 3 + a
                rbase = off + a * hw

                # ---- quad path (bulk) ----
                for sg in qsgroups:
                    nq = len(sg)
                    stg = stage_pool.tile([128, 340 * nq], F32, tag="stage")
                    stg4 = stg[:].rearrange("p (q e c) -> p q e c", e=4,
                                            c=ATTRS)
                    for qi, bi in enumerate(sg):
                        st = quads[bi]
                        pm = psum_main.tile([128, 340], F32, tag="pmain")
                        for e in range(4):
                            nc.tensor.transpose(
                                pm[:, ATTRS * e:ATTRS * (e + 1)],
                                _strided_cols(ut[:], colbase + st, 4, e),
                                ident[0:85, 0:85])
                        pm3 = pm[:].rearrange("p (e c) -> p e c", c=ATTRS)
                        sl = stg4[:, qi, :, :]
                        nc.vector.tensor_copy(sl[:, :, 4:ATTRS],
                                              pm3[:, :, 4:ATTRS])
                        bsrc = bstages[bi][:].rearrange(
                            "p (e k j) -> p e k j", k=4, j=12)
                        nc.vector.tensor_copy(sl[:, :, 0:4],
                                              bsrc[:, :, :, j])
                    r0 = rbase + quads[sg[0]]
                    dst = out[b, r0:r0 + nq * 512, :] \
                        .rearrange("(q p e) c -> p q e c", p=128, e=4)
                    nc.scalar.dma_start(dst, stg4[:, 0:nq, :, :])

                # ---- tail path (single chunks) ----
                for tgi, grp in enumerate(tail_groups):
                    n = len(grp)
                    pm = psum_main.tile([128, ATTRS * n], F32, tag="pmain",
                                        name="pm_tail")
                    for q, st in enumerate(grp):
                        nc.tensor.transpose(
                            pm[:, ATTRS * q:ATTRS * (q + 1)],
                            ut[:, colbase + st:colbase + st + 128],
                            ident[0:85, 0:85])
                    stg = stage_pool.tile([128, ATTRS * n], F32, tag="stage",
                                          name="stg_tail")
                    stg3 = stg[:].rearrange("p (q c) -> p q c", c=ATTRS)
                    pm3 = pm[:].rearrange("p (q c) -> p q c", c=ATTRS)
                    nc.vector.tensor_copy(stg3[:, :, 4:ATTRS],
                                          pm3[:, :, 4:ATTRS])
                    bsrc = bstages[nquad_blocks + tgi][:].rearrange(
                        "p (q k j) -> p q k j", k=4, j=12)
                    nc.vector.tensor_copy(stg3[:, :, 0:4], bsrc[:, :, :, j])
                    for qoff, nrun, st0 in _runs(grp, 128):
                        dst = out[b, rbase + st0:rbase + st0 + nrun * 128, :] \
                            .rearrange("(q p) c -> p q c", p=128)
                        nc.scalar.dma_start(dst, stg3[:, qoff:qoff + nrun, :])


_NC_CACHE = []


def _get_nc():
    if not _NC_CACHE:
        _NC_CACHE.append(build_nc())
    return _NC_CACHE[0]


def kernel(feat0, feat1, feat2):
    feats = [np.ascontiguousarray(np.asarray(f, dtype=np.float32))
             for f in (feat0, feat1, feat2)]
    assert feats[0].shape == (B_FULL, 255, 19, 19)
    assert feats[1].shape == (B_FULL, 255, 38, 38)
    assert feats[2].shape == (B_FULL, 255, 76, 76)

    consts = host_consts()
    nc = _get_nc()
    in_maps = []
    for c in range(N_CORES):
        m = dict(consts)
        for s in range(3):
            m[f"feat{s}"] = feats[s][c * B_LOCAL:(c + 1) * B_LOCAL]
        in_maps.append(m)

    res = run_bass_kernel_spmd(nc, in_maps, list(range(N_CORES)))
    return np.concatenate([res.results[c]["out"] for c in range(N_CORES)],
                          axis=0)
